# revision 1
# baseline (speedup 1.0000x reference)
"""Trainium2 Bass kernel for nn_Block_3822520894096 (dense transformer block).

Strategy: data-parallel over batch B=32 across 8 NeuronCores (4 images/core,
all params replicated). Inside each core everything runs as fp32r matmuls on
the PE with softmax on ACT/DVE:

  - qkv 1x1 conv        -> PE matmuls (q,k normal orientation; v produced
                           transposed with an extra "ones" column so the
                           attention value-matmul also emits softmax row sums)
  - depthwise 3x3       -> 9 PE matmuls with per-tap diagonal weight matrices
                           over guard-padded flat rows
  - rel-pos bias        -> gather is pure indexing done host-side (no FLOPs);
                           the two bicubic-interp matmuls run on PE once per
                           head; result is ADDED to q.k logits for free by
                           augmenting the contraction with an identity block
  - softmax             -> exp on ACT (no max subtraction needed: logits are
                           O(10)), denominator from the ones column, divide
                           folded into the o epilogue
  - proj                -> accumulated per head in PSUM as heads finish
  - ffn                 -> PE matmuls, BN affine + residual on DVE/ACT

kernel(**inputs) takes FULL unsharded inputs and returns the FULL output.
"""

import os
import sys
import numpy as np

sys.path.insert(0, "/opt/trn_rl_repo")

import concourse.bass as bass  # noqa: E402
import concourse.tile as tile  # noqa: E402
from concourse import bacc, mybir  # noqa: E402
from contextlib import ExitStack  # noqa: E402

# ---------------------------------------------------------------- constants
B, C, HH, WW = 32, 256, 20, 20
N = HH * WW              # 400 pixels
NH, KD = 8, 16           # heads, per-head qk dim
D = 64                   # per-head v dim
DH = NH * D              # 512
S = 196                  # native bias grid (14*14)
SCALE = KD ** -0.5
NCORES = 8
BL = B // NCORES         # local batch = 4

P98, P100 = 98, 100
F32 = mybir.dt.float32
MM_DT = mybir.dt.float32r   # fast fp32 matmul path on PE


def _bicubic_matrix(out_n, in_n):
    # torch F.interpolate(mode='bicubic', align_corners=False), dense matrix.
    a = -0.75
    M = np.zeros((out_n, in_n), np.float64)
    scale = in_n / out_n
    for i in range(out_n):
        src = (i + 0.5) * scale - 0.5
        f = int(np.floor(src))
        t = src - f
        for j in range(-1, 3):
            xx = abs(j - t)
            if xx <= 1.0:
                w = (a + 2) * xx**3 - (a + 3) * xx**2 + 1
            elif xx < 2.0:
                w = a * xx**3 - 5 * a * xx**2 + 8 * a * xx - 4 * a
            else:
                w = 0.0
            M[i, min(max(f + j, 0), in_n - 1)] += w
    return M.astype(np.float32)


def _wt_dev(w_t, pchunk=128):
    """[K, M] (K contraction) -> sbuf layout [pchunk, K//pchunk, M]."""
    K, M = w_t.shape
    return np.ascontiguousarray(
        w_t.reshape(K // pchunk, pchunk, M).transpose(1, 0, 2)
    )


def _build_kernel():
    nc = bacc.Bacc(
        "TRN2", target_bir_lowering=False, debug=False, num_devices=NCORES
    )

    def din(name, shape, dt=F32):
        return nc.dram_tensor(name, list(shape), dt, kind="ExternalInput").ap()

    x_d = din("x", (BL, 128, 2, N), MM_DT)   # [b, part, cchunk, 400]
    wqkT_d = din("wqkT", (128, 2, 256), MM_DT)
    wvT_d = din("wvT", (128, 2, 512), MM_DT)
    wprojT_d = din("wprojT", (64, NH, 256), MM_DT)
    wpw1T_d = din("wpw1T", (128, 2, 512), MM_DT)
    wpw2T_d = din("wpw2T", (128, 4, 256), MM_DT)
    g_d = din("gbias", (P98, 2, NH, S), MM_DT)  # gathered bias [s_p, sc, h, t]
    mt_d = din("mt", (P98, 2, N), MM_DT)     # M.T            [t_p, tc, n]
    eye4_d = din("eye4", (P100, N), MM_DT)   # [I I I I]
    eye128_d = din("eye128", (128, 128))
    dww_d = din("dww", (128, 9))
    qsq_d = din("qs_q", (128, 1))
    qbq_d = din("qb_q", (128, 1))
    qsk_d = din("qs_k", (128, 1))
    qbk_d = din("qb_k", (128, 1))
    sv_d = din("sv", (64, NH))
    bv_d = din("bv", (64, NH))
    dws_d = din("dws", (128, 1))
    dwb_d = din("dwb", (128, 1))
    ps_d = din("ps", (128, 2))
    pb_d = din("pb", (128, 2))
    p1s_d = din("p1s", (128, 4))
    p1b_d = din("p1b", (128, 4))
    p2s_d = din("p2s", (128, 2))
    p2b_d = din("p2b", (128, 2))

    y_d = nc.dram_tensor("y", [BL, 128, 2, N], F32, kind="ExternalOutput").ap()

    AF = mybir.ActivationFunctionType
    ALU = mybir.AluOpType

    def mmc(ap):
        return ap.bitcast(MM_DT)

    with tile.TileContext(nc) as tc, ExitStack() as ctx:
        sing = ctx.enter_context(tc.tile_pool(name="sing", bufs=1))

        def load(nm, d_ap, shape, dt=F32):
            t = sing.tile(list(shape), dt, name=nm, tag=nm)
            nc.sync.dma_start(t[:], d_ap)
            return t

        wqkT = load("wqkT", wqkT_d, (128, 2, 256), MM_DT)
        wvT = load("wvT", wvT_d, (128, 2, 512), MM_DT)
        wprojT = load("wprojT", wprojT_d, (64, NH, 256), MM_DT)
        wpw1T = load("wpw1T", wpw1T_d, (128, 2, 512), MM_DT)
        wpw2T = load("wpw2T", wpw2T_d, (128, 4, 256), MM_DT)
        eye128 = load("eye128", eye128_d, (128, 128))
        dww = load("dwwc", dww_d, (128, 9))
        qsq = load("qsq", qsq_d, (128, 1))
        qbq = load("qbq", qbq_d, (128, 1))
        qsk = load("qsk", qsk_d, (128, 1))
        qbk = load("qbk", qbk_d, (128, 1))
        sv = load("svc", sv_d, (64, NH))
        bv = load("bvc", bv_d, (64, NH))
        dws = load("dwsc", dws_d, (128, 1))
        dwb = load("dwbc", dwb_d, (128, 1))
        ps = load("psc", ps_d, (128, 2))
        pb = load("pbc", pb_d, (128, 2))
        p1s = load("p1sc", p1s_d, (128, 4))
        p1b = load("p1bc", p1b_d, (128, 4))
        p2s = load("p2sc", p2s_d, (128, 2))
        p2b = load("p2bc", p2b_d, (128, 2))

        # k-side BN affine folded with attention SCALE (device, tiny)
        qsk_s = sing.tile([128, 1], F32)
        qbk_s = sing.tile([128, 1], F32)
        nc.vector.tensor_scalar_mul(qsk_s[:], qsk[:], SCALE)
        nc.vector.tensor_scalar_mul(qbk_s[:], qbk[:], SCALE)

        # diagonal depthwise weight matrices [128, tap, 128]
        dwdiag = sing.tile([128, 9, 128], MM_DT)
        for tap in range(9):
            nc.vector.tensor_scalar_mul(
                dwdiag[:, tap, :], eye128[:], dww[:, tap : tap + 1]
            )

        # Attention operand buffers (shared across b; PE runs in order so the
        # per-b k/q rewrites pipeline fine):
        #   lb[0:100, h, :]   = [I I I I] (same for every head)
        #   lb[100:116, h, :] = k_h            (one DMA per image)
        #   rb[0:100, h, kc, :]   = R_T[h] key-chunk kc   (interp, once)
        #   rb[100:116, h, kc, :] = q_h  (replicated over kc; one DMA/image)
        lb = sing.tile([116, NH, N], MM_DT, name="lb", tag="lb")
        nc.sync.dma_start(
            lb[0:P100, :, :],
            eye4_d.unsqueeze(1).broadcast_to((P100, NH, N)),
        )
        rb = sing.tile([116, 4, NH, N], MM_DT, name="rb", tag="rb")

        # ---------------- pools
        psAt = ctx.enter_context(tc.tile_pool(name="psAt", bufs=2, space="PSUM"))
        psPj = ctx.enter_context(tc.tile_pool(name="psPj", bufs=2, space="PSUM"))
        psMm = ctx.enter_context(tc.tile_pool(name="psMm", bufs=2, space="PSUM"))
        sb3 = ctx.enter_context(tc.tile_pool(name="sb3", bufs=4))
        qk_pool = ctx.enter_context(tc.tile_pool(name="qk", bufs=2))
        vt_pool = ctx.enter_context(tc.tile_pool(name="vt", bufs=2))
        ex_pool = ctx.enter_context(tc.tile_pool(name="ex", bufs=2))
        oh_pool = ctx.enter_context(tc.tile_pool(name="oh", bufs=4))
        sm_pool = ctx.enter_context(tc.tile_pool(name="sm", bufs=2))
        dram_pool = ctx.enter_context(
            tc.tile_pool(name="drb", bufs=2, space="DRAM")
        )

        # ---------------- rel-pos bias interpolation (once per head)
        # stage 1: Q1[t, n] = sum_s G[s, t] * M[n, s]
        # stage 2: R_T[key, n] = sum_t M[key, t] Q1[t, n] -> rb[0:100, h, kc]
        with tc.tile_pool(name="interp_sb", bufs=2) as interp_sb:
            gb = interp_sb.tile([P98, 2, NH, S], MM_DT, name="gb", tag="gb", bufs=1)
            nc.sync.dma_start(gb[:], g_d)
            mt = interp_sb.tile([P98, 2, N], MM_DT, name="mtc", tag="mtc", bufs=1)
            nc.sync.dma_start(mt[:], mt_d)
            q1s = {}

            def interp_s1(h):
                q1 = interp_sb.tile([P98, 2, N], MM_DT, tag="q1", bufs=2)
                q1s[h] = q1
                for tci in range(2):
                    p1 = psMm.tile([P98, N], F32, tag="mm", name="p1")
                    for sc in range(2):
                        nc.tensor.matmul(
                            p1[:],
                            mmc(gb[0:P98, sc, h, tci * P98 : (tci + 1) * P98]),
                            mmc(mt[0:P98, sc, :]),
                            start=(sc == 0),
                            stop=(sc == 1),
                        )
                    nc.scalar.copy(q1[:, tci, :], p1[:])

            def interp_s2(h):
                q1 = q1s.pop(h)
                for kc in range(4):
                    p2 = psMm.tile([P100, N], F32, tag="mm", name="p2")
                    for tci in range(2):
                        nc.tensor.matmul(
                            p2[:],
                            mmc(mt[0:P98, tci, kc * P100 : (kc + 1) * P100]),
                            mmc(q1[0:P98, tci, :]),
                            start=(tci == 0),
                            stop=(tci == 1),
                        )
                    nc.vector.tensor_copy(rb[0:P100, kc, h, :], p2[:])

            interp_s1(0)
            for h in range(NH):
                if h + 1 < NH:
                    interp_s1(h + 1)
                interp_s2(h)

        # ---------------- per-image software-pipelined emission
        # PE executes its stream in order, so emission order IS the PE
        # schedule: attention matmuls for unit i run while unit i-1's exp
        # output feeds the value-matmul and unit i-2's proj accumulates,
        # hiding the cross-engine latencies (exp, softmax divide, epilogues).
        WP = WW + 1
        NP = HH * WP
        GP = 22
        st = {}

        def emit_prologue(b):
            s = {}
            x_sb = sm_pool.tile([128, 2, N], MM_DT, tag="x", name=f"x{b}")
            nc.sync.dma_start(x_sb[:], x_d[b])
            s["x"] = x_sb
            qpre = qk_pool.tile([128, GP + NP + GP], MM_DT, tag="qpre")
            nc.vector.memset(qpre[:].bitcast(F32), 0.0)
            qpre_rows = qpre[:, GP : GP + NP].rearrange(
                "p (a b) -> p a b", a=HH
            )
            k_sb = qk_pool.tile([128, N], F32, tag="ksb")
            for mc in range(2):
                pqk = psMm.tile([128, N], F32, tag="mm", name="pqk")
                for kci in range(2):
                    nc.tensor.matmul(
                        pqk[:],
                        mmc(wqkT[:, kci, mc * 128 : (mc + 1) * 128]),
                        mmc(x_sb[:, kci, :]),
                        start=(kci == 0),
                        stop=(kci == 1),
                    )
                if mc == 0:
                    nc.vector.tensor_scalar(
                        qpre_rows[:, :, 0:WW],
                        pqk[:].rearrange("p (a b) -> p a b", a=HH),
                        qsq[:], qbq[:], ALU.mult, ALU.add,
                    )
                else:
                    nc.vector.tensor_scalar(
                        k_sb[:], pqk[:], qsk_s[:], qbk_s[:], ALU.mult, ALU.add
                    )
            # depthwise 3x3 (9 diagonal matmuls on flat padded rows)
            pdw = psMm.tile([128, NP], F32, tag="mm", name="pdw")
            taps = [(0, 0)] + [
                (dy, dx) for dy in (-1, 0, 1) for dx in (-1, 0, 1)
                if (dy, dx) != (0, 0)
            ]
            for ti, (dy, dx) in enumerate(taps):
                wi = (dy + 1) * 3 + (dx + 1)
                off = dy * WP + dx
                nc.tensor.matmul(
                    pdw[:],
                    mmc(dwdiag[:, wi, :]),
                    mmc(qpre[:, GP + off : GP + off + NP]),
                    start=(ti == 0),
                    stop=(ti == len(taps) - 1),
                )
            q_sb = qk_pool.tile([128, N], F32, tag="qsb")
            nc.vector.tensor_scalar(
                q_sb[:].rearrange("p (a b) -> p a b", a=HH),
                pdw[:].rearrange("p (a b) -> p a b", a=HH)[:, :, 0:WW],
                dws[:], dwb[:], ALU.mult, ALU.add,
            )
            # regroup k,q to [d, h, n] partitions via DRAM bounce
            kdram = dram_pool.tile([16, NH, N], MM_DT, tag="kdram")
            qdram = dram_pool.tile([16, NH, N], MM_DT, tag="qdram")
            kdst = bass.AP(
                tensor=kdram.tensor, offset=kdram[:].offset,
                ap=[[N, NH], [NH * N, 16], [1, N]],
            )
            nc.sync.dma_start(kdst, k_sb[:].bitcast(MM_DT))
            qdst = bass.AP(
                tensor=qdram.tensor, offset=qdram[:].offset,
                ap=[[N, NH], [NH * N, 16], [1, N]],
            )
            nc.sync.dma_start(qdst, q_sb[:].bitcast(MM_DT))
            s["kdram"] = kdram
            s["qdram"] = qdram
            # v transposed with ones column
            vt = vt_pool.tile([P100, 4, NH, 65], MM_DT, tag="vt")
            nc.vector.memset(vt[:, :, :, 64].bitcast(F32), 1.0)
            for qc in range(4):
                pv = psMm.tile([P100, 512], F32, tag="mm", name="pv")
                for kci in range(2):
                    nc.tensor.matmul(
                        pv[:],
                        mmc(x_sb[:, kci, qc * P100 : (qc + 1) * P100]),
                        mmc(wvT[:, kci, :]),
                        start=(kci == 0),
                        stop=(kci == 1),
                    )
                nc.vector.tensor_copy(
                    vt[:, qc, :, 0:64],
                    pv[:].rearrange("p (a b) -> p a b", a=NH),
                )
            s["vt"] = vt
            s["ex"] = {}
            s["oh"] = {}
            return s

        def emit_attn(b, h):
            s = st[b]
            ex = ex_pool.tile([P100, 4, N], MM_DT, tag="ex")
            s["ex"][h] = ex
            for pair in range(2):
                pat = psAt.tile([P100, 2, 512], F32, tag="at")
                for j in range(2):
                    kc = pair * 2 + j
                    nc.tensor.matmul(
                        pat[:, j, 0:N],
                        mmc(lb[0:116, h, kc * P100 : (kc + 1) * P100]),
                        mmc(rb[0:116, kc, h, :]),
                        start=True,
                        stop=True,
                    )
                nc.scalar.activation(
                    ex[:, pair * 2 : pair * 2 + 2, :],
                    pat[:, :, 0:N],
                    AF.Exp,
                )

        def emit_o(b, h):
            s = st[b]
            ex = s["ex"].pop(h)
            vt = s["vt"]
            po = psMm.tile([65, 512], F32, tag="mm", name="po")
            for kc in range(4):
                nc.tensor.matmul(
                    po[:, 0:N],
                    mmc(vt[:, kc, h, :]),
                    mmc(ex[:, kc, :]),
                    start=(kc == 0),
                    stop=(kc == 3),
                )
            r_sb = sb3.tile([1, N], F32, tag="r")
            nc.vector.reciprocal(r_sb[:], po[64:65, 0:N])
            r64 = sb3.tile([64, N], F32, tag="r64")
            nc.gpsimd.partition_broadcast(r64[:], r_sb[:])
            o_tmp = sb3.tile([64, N], F32, tag="otmp")
            nc.vector.tensor_tensor(o_tmp[:], po[0:64, 0:N], r64[:], ALU.mult)
            o_h = oh_pool.tile([64, N], MM_DT, tag="oh")
            nc.scalar.activation(
                o_h[:], o_tmp[:], AF.Relu,
                bias=bv[:, h : h + 1], scale=sv[:, h : h + 1],
            )
            s["oh"][h] = o_h

        def emit_pj(b, h):
            s = st[b]
            if h == 0:
                s["pj"] = [
                    psPj.tile([128, N], F32, tag="pj", name=f"pj{b}_{m}")
                    for m in range(2)
                ]
            o_h = s["oh"].pop(h)
            for mc, pj in enumerate(s["pj"]):
                nc.tensor.matmul(
                    pj[:],
                    mmc(wprojT[0:64, h, mc * 128 : (mc + 1) * 128]),
                    mmc(o_h[:]),
                    start=(h == 0),
                    stop=(h == NH - 1),
                )

        def emit_ffn(b):
            s = st.pop(b)
            x_sb = s["x"]
            x2 = sm_pool.tile([128, 2, N], F32, tag="x2")
            x2r = sm_pool.tile([128, 2, N], MM_DT, tag="x2r", bufs=1)
            for mc, pj in enumerate(s["pj"]):
                nc.vector.tensor_scalar(
                    pj[:], pj[:], ps[:, mc : mc + 1], pb[:, mc : mc + 1],
                    ALU.mult, ALU.add,
                )
                nc.vector.tensor_tensor(
                    x2[:, mc, :], pj[:], x_sb[:, mc, :].bitcast(F32), ALU.add
                )
            nc.vector.tensor_copy(x2r[:], x2[:].bitcast(MM_DT))
            hsb = sm_pool.tile([128, 4, N], MM_DT, tag="hsb", bufs=1)
            for mc in range(4):
                p1m = psMm.tile([128, N], F32, tag="mm", name="p1m")
                for kci in range(2):
                    nc.tensor.matmul(
                        p1m[:],
                        mmc(wpw1T[:, kci, mc * 128 : (mc + 1) * 128]),
                        mmc(x2r[:, kci, :]),
                        start=(kci == 0),
                        stop=(kci == 1),
                    )
                nc.scalar.activation(
                    hsb[:, mc, :], p1m[:], AF.Relu,
                    bias=p1b[:, mc : mc + 1], scale=p1s[:, mc : mc + 1],
                )
            out_sb = sm_pool.tile([128, 2, N], F32, tag="out")
            for mc in range(2):
                p2m = psMm.tile([128, N], F32, tag="mm", name="p2m")
                for kci in range(4):
                    nc.tensor.matmul(
                        p2m[:],
                        mmc(wpw2T[:, kci, mc * 128 : (mc + 1) * 128]),
                        mmc(hsb[:, kci, :]),
                        start=(kci == 0),
                        stop=(kci == 3),
                    )
                nc.vector.tensor_scalar(
                    p2m[:], p2m[:], p2s[:, mc : mc + 1], p2b[:, mc : mc + 1],
                    ALU.mult, ALU.add,
                )
                nc.vector.tensor_tensor(
                    out_sb[:, mc, :], p2m[:], x2[:, mc, :], ALU.add
                )
            nc.sync.dma_start(y_d[b], out_sb[:])

        def emit_kq_load(b):
            # Traced at the image boundary: lb/rb row rewrites must come
            # after the previous image's attention matmuls in trace order.
            s = st[b]
            nc.sync.dma_start(lb[P100 : P100 + 16, :, :], s.pop("kdram")[:])
            qd = s.pop("qdram")
            for kc in range(4):
                nc.sync.dma_start(rb[P100 : P100 + 16, kc, :, :], qd[:])

        units = [(b, h) for b in range(BL) for h in range(NH)]
        st[0] = emit_prologue(0)
        emit_kq_load(0)
        n_u = len(units)
        for i in range(n_u + 3):
            boundary = i < n_u and i > 0 and units[i][1] == 0
            if boundary:
                b, h = units[i]
                emit_kq_load(b)
                emit_o(*units[i - 1])
                emit_pj(*units[i - 2])
                emit_attn(b, h)
            else:
                if i < n_u:
                    b, h = units[i]
                    emit_attn(b, h)
                if 1 <= i < n_u + 1:
                    emit_o(*units[i - 1])
                if 2 <= i < n_u + 2:
                    emit_pj(*units[i - 2])
            if i < n_u and units[i][1] == 4 and units[i][0] + 1 < BL:
                st[units[i][0] + 1] = emit_prologue(units[i][0] + 1)
            if 3 <= i and units[i - 3][1] == NH - 1:
                emit_ffn(units[i - 3][0])

    nc.compile()
    return nc


_CACHE = {}


def _prep_inputs(inputs):
    """Host prep: sharding + pure relayout/indexing (no float math)."""
    x = np.ascontiguousarray(
        np.asarray(inputs["x"], np.float32)
        .reshape(B, 2, 128, N)
        .transpose(0, 2, 1, 3)
    )  # [b, part, cchunk, n]
    qkv_w = np.asarray(inputs["qkv_w"], np.float32)
    qkv_s = np.asarray(inputs["qkv_s"], np.float32)
    qkv_b = np.asarray(inputs["qkv_b"], np.float32)
    dw_w = np.asarray(inputs["dw_w"], np.float32)
    g = np.asarray(inputs["attn_biases"], np.float32)[
        :, np.asarray(inputs["bias_idxs"])
    ]  # [8, 196, 196] pure gather
    M = _bicubic_matrix(N, S)
    mt = np.ascontiguousarray(M.T.reshape(2, P98, N).transpose(1, 0, 2))
    gdev = np.ascontiguousarray(
        g.reshape(NH, 2, P98, S).transpose(2, 1, 0, 3)
    )  # [98, sc, h, t]

    common = dict(
        wqkT=_wt_dev(np.ascontiguousarray(qkv_w[0:256].T)),
        wvT=_wt_dev(np.ascontiguousarray(qkv_w[256:768].T)),
        wprojT=_wt_dev(
            np.ascontiguousarray(np.asarray(inputs["proj_w"], np.float32).T),
            pchunk=64,
        ),
        wpw1T=_wt_dev(np.ascontiguousarray(np.asarray(inputs["pw1_w"], np.float32).T)),
        wpw2T=_wt_dev(np.ascontiguousarray(np.asarray(inputs["pw2_w"], np.float32).T)),
        gbias=gdev,
        mt=mt,
        eye4=np.tile(np.eye(P100, dtype=np.float32), (1, 4)),
        eye128=np.eye(128, dtype=np.float32),
        dww=np.ascontiguousarray(dw_w.reshape(128, 9)),
        qs_q=qkv_s[0:128, None],
        qb_q=qkv_b[0:128, None],
        qs_k=qkv_s[128:256, None],
        qb_k=qkv_b[128:256, None],
        sv=np.ascontiguousarray(qkv_s[256:768].reshape(NH, 64).T),
        bv=np.ascontiguousarray(qkv_b[256:768].reshape(NH, 64).T),
        dws=np.asarray(inputs["dw_s"], np.float32)[:, None],
        dwb=np.asarray(inputs["dw_b"], np.float32)[:, None],
        ps=np.ascontiguousarray(np.asarray(inputs["proj_s"], np.float32).reshape(2, 128).T),
        pb=np.ascontiguousarray(np.asarray(inputs["proj_b"], np.float32).reshape(2, 128).T),
        p1s=np.ascontiguousarray(np.asarray(inputs["pw1_s"], np.float32).reshape(4, 128).T),
        p1b=np.ascontiguousarray(np.asarray(inputs["pw1_b"], np.float32).reshape(4, 128).T),
        p2s=np.ascontiguousarray(np.asarray(inputs["pw2_s"], np.float32).reshape(2, 128).T),
        p2b=np.ascontiguousarray(np.asarray(inputs["pw2_b"], np.float32).reshape(2, 128).T),
    )
    common = {k: np.ascontiguousarray(v, np.float32) for k, v in common.items()}
    in_maps = []
    for c in range(NCORES):
        m = dict(common)
        m["x"] = np.ascontiguousarray(x[c * BL : (c + 1) * BL])
        in_maps.append(m)
    return in_maps


def kernel(**inputs):
    from concourse.bass_utils import run_bass_kernel_spmd

    if "nc" not in _CACHE:
        _CACHE["nc"] = _build_kernel()
    nc = _CACHE["nc"]
    in_maps = _prep_inputs(inputs)
    res = run_bass_kernel_spmd(nc, in_maps, list(range(NCORES)))
    y = np.concatenate([r["y"] for r in res.results], axis=0)  # [32,128,2,400]
    y = y.transpose(0, 2, 1, 3)  # [32, 2, 128, 400]
    return np.ascontiguousarray(y.reshape(B, C, HH, WW))

